# revision 34
# baseline (speedup 1.0000x reference)
"""Trainium2 distributed Bass kernel for the APGAT layer (gnn_message_passing).

Strategy (8 NeuronCores):
  - Sort edges by dst; shard dst rows (10000/8 = 1250 per core) so each pred
    node's softmax + aggregation is fully local to one core.
  - Phase A: shard the z = h @ W_fc.T matmul over nodes (6272/core, bf16).
    The matmul rhs is widened to 264 cols: [W_fc.T | Wz_eff.T], where Wz_eff
    folds a_src, so the per-node attention score s_src rides in the same
    768-byte row. The [z | s_src] table is distributed in TWO AllGathers
    (node tiles 0:25 -> table A, 25:49 -> table B) so half-A gathers can
    start while the second AllGather is still in flight.
  - Phase B: per core, 10 windows of 128 dst rows. Window edges are split by
    which table their src row lives in (both tables < 32768 rows so gather
    indices fit int16), each group padded to a fixed capacity C_half.
    Per half:
      * dma_gather [z | s_src] rows (768 B) in <=896-index chunks
        (single_packet needs <=64 ring descriptors), round-robined over
        4 SWDGE queues so multiple gathers drain concurrently
      * s_feat via PE (srl tile stationary, Wf_eff.T moving)
      * ex = exp(leaky_relu(s_src + s_feat))
      * msg = ex (x) z  (DVE broadcast multiply)
      * segment-sum via one-hot matmul: lhsT = is_equal(iota, dst_rel),
        rhs = [msg | ex]; PSUM accumulates [128 dst, 256 msg + 8 denom].
    Finalize: out = msg_sum / max(denom, tiny) -> output shard.

Softmax max-subtraction is skipped (scores are O(7); verified ~4.6e-3 rel
err vs the reference, gate is 2e-2).
"""

import sys

sys.path.insert(0, "/opt/trn_rl_repo")

import numpy as np
import ml_dtypes

import concourse.bass as bass
import concourse.bacc as bacc
import concourse.mybir as mybir
import concourse.tile as tile
from concourse.tile import add_dep_helper
from concourse.bass_utils import run_bass_kernel_spmd

BF16 = ml_dtypes.bfloat16
F32 = np.float32
AF = mybir.ActivationFunctionType
ALU = mybir.AluOpType

GATHER_CHUNK = 896   # <=64 ring descriptors per single-packet gather
N_SWDGE_Q = 4


class Cfg:
    def __init__(self, ncores=8, na=50000, np_=10000, e=400000, in_dim=512,
                 feat=128, h=8, d=32, nw=10, c_half=2816):
        self.NCORES = ncores
        self.NA = na
        self.NP = np_
        self.E = e
        self.IN_DIM = in_dim
        self.KC = in_dim // 128           # k chunks for z matmul
        self.FEAT = feat
        self.H = h
        self.D = d
        self.HD = h * d                   # 256
        self.HDE = self.HD + h            # 264: [z | s_src]
        self.RW = (-(-(self.HDE * 2) // 256) * 256) // 2  # 384 bf16 = 768 B rows
        self.NW = nw
        self.WIN = 128
        self.DST_PER_CORE = np_ // ncores
        assert self.DST_PER_CORE <= nw * 128
        nsh = -(-na // ncores)
        self.NSH = -(-nsh // 128) * 128   # nodes per core (6272)
        self.NA_PAD = self.NSH * ncores
        self.NT = self.NSH // 128         # node tiles per core (49)
        # split node tiles into two chunks == the two gather tables
        self.NT_A = -(-self.NT // 2)      # 25
        self.NT_B = self.NT - self.NT_A   # 24
        self.ROWS_A = self.NT_A * 128     # per-core rows in table A (3200)
        self.ROWS_B = self.NT_B * 128     # (3072)
        self.TBL_A = self.ROWS_A * ncores  # 25600
        self.TBL_B = self.ROWS_B * ncores  # 24576
        assert max(self.TBL_A, self.TBL_B) <= 32768  # int16 gather indices
        self.C_HALF = c_half              # per-window per-table capacity
        assert c_half % 128 == 0
        self.T_HALF = c_half // 128
        self.W_CAP = 2 * c_half
        self.T_WIN = 2 * self.T_HALF
        self.E_CAP = nw * self.W_CAP
        self.OUT_ROWS = nw * 128


def _gather_chunks(c_half):
    out, pos = [], 0
    while pos < c_half:
        n = min(GATHER_CHUNK, c_half - pos)
        out.append((pos, n))
        pos += n
    return out


def build_graph(cfg: Cfg):
    """Build the SPMD Bass graph (same graph runs on every core)."""
    nc = bacc.Bacc("TRN2", target_bir_lowering=False, debug=False,
                   num_devices=cfg.NCORES, num_swdge_queues=N_SWDGE_Q)
    bf = mybir.dt.bfloat16
    f32 = mybir.dt.float32
    i16 = mybir.dt.int16

    # ---- kernel I/O ----
    hT = nc.dram_tensor("hT", [128, cfg.KC, cfg.NSH], bf, kind="ExternalInput")
    WfcT = nc.dram_tensor("WfcT", [128, cfg.KC, cfg.HDE], bf, kind="ExternalInput")
    WfeT = nc.dram_tensor("WfeT", [cfg.FEAT, cfg.H], bf, kind="ExternalInput")
    IotaM = nc.dram_tensor("IotaM", [128, 128], bf, kind="ExternalInput")
    srlT = nc.dram_tensor("srlT", [cfg.FEAT, cfg.E_CAP], bf, kind="ExternalInput")
    dstrel = nc.dram_tensor("dstrel", [128, cfg.E_CAP // 128], bf, kind="ExternalInput")
    idxA = nc.dram_tensor("idxA", [128, cfg.NW * (cfg.C_HALF // 16)], i16, kind="ExternalInput")
    idxB = nc.dram_tensor("idxB", [128, cfg.NW * (cfg.C_HALF // 16)], i16, kind="ExternalInput")
    n_chunks = len(_gather_chunks(cfg.C_HALF))
    gcnt = nc.dram_tensor("gcnt", [1, cfg.NW * 2 * n_chunks], mybir.dt.int32,
                          kind="ExternalInput")
    out_ext = nc.dram_tensor("out", [cfg.OUT_ROWS, cfg.HD], f32, kind="ExternalOutput")

    groups = [list(range(cfg.NCORES))]
    shared = "Shared" if cfg.NCORES > 4 else "Local"

    with tile.TileContext(nc) as tc:
        with (
            tc.tile_pool(name="dram", bufs=1, space="DRAM") as dram,
            tc.tile_pool(name="consts", bufs=1) as consts,
            tc.tile_pool(name="psum_z", bufs=2, space="PSUM") as psum_z,
            tc.tile_pool(name="psum_sf", bufs=2, space="PSUM") as psum_sf_pool,
            tc.tile_pool(name="psum_acc", bufs=2, space="PSUM") as psum_acc_pool,
            tc.tile_pool(name="srl", bufs=2) as srl_pool,
            tc.tile_pool(name="zg", bufs=2) as zg_pool,
            tc.tile_pool(name="msg", bufs=2) as msg_pool,
            tc.tile_pool(name="oh", bufs=2) as oh_pool,
            tc.tile_pool(name="acca", bufs=8) as acca_pool,
            tc.tile_pool(name="sfst", bufs=6) as sfst_pool,
            tc.tile_pool(name="small", bufs=3) as small,
            tc.tile_pool(name="outst", bufs=2) as outst,
        ):
            # ---- constants / index streams (loaded once) ----
            wfe_sb = consts.tile([cfg.FEAT, cfg.H], bf)
            nc.sync.dma_start(wfe_sb[:], WfeT[:])
            iota_sb = consts.tile([128, 128], bf)
            nc.sync.dma_start(iota_sb[:], IotaM[:])
            idxa_sb = consts.tile([128, cfg.NW * (cfg.C_HALF // 16)], i16)
            nc.sync.dma_start(idxa_sb[:], idxA[:])
            idxb_sb = consts.tile([128, cfg.NW * (cfg.C_HALF // 16)], i16)
            nc.sync.dma_start(idxb_sb[:], idxB[:])
            dst_sb = consts.tile([128, cfg.E_CAP // 128], bf)
            nc.sync.dma_start(dst_sb[:], dstrel[:])
            gcnt_sb = consts.tile([1, cfg.NW * 2 * n_chunks], mybir.dt.int32)
            nc.sync.dma_start(gcnt_sb[:], gcnt[:])

            # ---- phase A: [z | s_src] shard matmuls -> two AllGathers ----
            z_out_a = dram.tile([cfg.TBL_A, cfg.RW], bf, addr_space=shared)
            z_out_b = dram.tile([cfg.TBL_B, cfg.RW], bf, addr_space=shared)
            z_in_a = dram.tile([cfg.ROWS_A, cfg.RW], bf)
            z_in_b = dram.tile([cfg.ROWS_B, cfg.RW], bf)

            ag_insts = []
            with tc.tile_pool(name="zphase", bufs=1) as zphase:
                hT_sb = zphase.tile([128, cfg.KC, cfg.NSH], bf)
                wfc_sb = zphase.tile([128, cfg.KC, cfg.HDE], bf)
                nc.sync.dma_start(wfc_sb[:], WfcT[:])
                for half, (t0, t1, z_in) in enumerate(
                        ((0, cfg.NT_A, z_in_a),
                         (cfg.NT_A, cfg.NT, z_in_b))):
                    nc.sync.dma_start(hT_sb[:, :, t0 * 128:t1 * 128],
                                      hT[:, :, t0 * 128:t1 * 128])
                    # chunk staged in SBUF so the Shared table gets exactly
                    # one writer instruction; the two chunk tiles share one
                    # slot (B reuses A's space after A's z_in DMA)
                    zstage = zphase.tile([128, cfg.NT_A, cfg.RW], bf,
                                         name=f"zstage{half}", tag="zstage")
                    if cfg.RW > cfg.HDE:
                        nc.vector.memset(zstage[:, :, cfg.HDE:cfg.RW], 0.0)
                    for nt in range(t0, t1):
                        pz = psum_z.tile([128, cfg.HDE], f32,
                                         name=f"pz{nt}", tag="pz")
                        for c in range(cfg.KC):
                            nc.tensor.matmul(
                                pz[:],
                                hT_sb[:, c, nt * 128:(nt + 1) * 128],
                                wfc_sb[:, c, :],
                                start=(c == 0), stop=(c == cfg.KC - 1),
                            )
                        nc.scalar.copy(zstage[:, nt - t0, 0:cfg.HDE], pz[:])
                    nc.sync.dma_start(
                        z_in[:].rearrange("(t p) r -> p t r", p=128),
                        zstage[:, 0:t1 - t0, :])
                    zo = z_out_a if half == 0 else z_out_b
                    ag = nc.gpsimd.collective_compute(
                        "AllGather", ALU.bypass,
                        ins=[z_in[:].opt()], outs=[zo[:].opt()],
                        replica_groups=groups)
                    ag_insts.append(ag.ins)

            # ---- phase B: edge windows, halves software-pipelined ----
            # Half-B gathers wait on the second AllGather; emitting them LAG
            # windows behind the half-A stream keeps the Pool engine free of
            # head-of-line blocking while AG_b is still in flight. Half-A
            # PSUM accumulators are evicted to SBUF right away so the lag
            # depth is not capped by PSUM banks.
            n_sf = cfg.T_HALF * cfg.H
            gq_counter = [0]
            accas = {}

            # zero the zg slots once: rows past a trimmed gather's count keep
            # stale slot data, which must be finite for the score/exp path
            zg_init = []
            for i in range(2):
                zgi = zg_pool.tile([128, cfg.T_HALF, cfg.RW], bf,
                                   name=f"zginit{i}", tag="zg")
                nc.vector.memset(zgi[:], 0.0)
                zg_init.append(zgi)
            del zg_init

            def emit_half(w, q):
                gq = 2 * w + q
                # per-half one-hot built from the window's dst_rel columns
                oh = oh_pool.tile([128, cfg.T_HALF, 128], bf,
                                  name=f"oh{gq}", tag=f"oh{q}")
                c0 = w * cfg.T_WIN + q * cfg.T_HALF
                nc.vector.tensor_tensor(
                    oh[:],
                    iota_sb[:].unsqueeze(1).broadcast_to([128, cfg.T_HALF, 128]),
                    dst_sb[:, c0:c0 + cfg.T_HALF]
                        .unsqueeze(2).broadcast_to([128, cfg.T_HALF, 128]),
                    ALU.is_equal,
                )
                pacc = psum_acc_pool.tile([128, cfg.HDE], f32,
                                          name=f"pacc{gq}", tag=f"pacc{q}")
                srl_sb = srl_pool.tile([cfg.FEAT, cfg.C_HALF], bf,
                                       name=f"srl{gq}", tag="srl")
                nc.sync.dma_start(
                    srl_sb[:], srlT[:, gq * cfg.C_HALF:(gq + 1) * cfg.C_HALF])

                # s_feat: stationary srl tile, moving Wf_eff.T -> [128e, 8].
                # Evicted to SBUF right away so this stream can run several
                # halves ahead of the gathers (fills the AllGather window).
                psf = psum_sf_pool.tile([128, cfg.T_HALF, cfg.H], f32,
                                        name=f"psf{gq}", tag="psf")
                for t in range(cfg.T_HALF):
                    nc.tensor.matmul(
                        psf[:, t, :],
                        srl_sb[:, t * 128:(t + 1) * 128],
                        wfe_sb[:],
                        start=True, stop=True,
                    )
                sf_sb = sfst_pool.tile([128, cfg.T_HALF, cfg.H], f32,
                                       name=f"sf{gq}", tag="sf")
                nc.scalar.copy(sf_sb[:], psf[:])

                # gather [z | s_src] rows for this half, in chunks
                zg = zg_pool.tile([128, cfg.T_HALF, cfg.RW], bf,
                                  name=f"zg{gq}", tag="zg")
                idx_sb = idxa_sb if q == 0 else idxb_sb
                tbl = z_out_a[:] if q == 0 else z_out_b[:]
                iw0 = w * (cfg.C_HALF // 16)
                for ci, (pos, n) in enumerate(_gather_chunks(cfg.C_HALF)):
                    # runtime count skips the trailing -1 padding indices
                    cnt = nc.gpsimd.alloc_register(f"gcnt{gq}_{ci}")
                    gi = gq * n_chunks + ci
                    nc.gpsimd.reg_load(cnt, gcnt_sb[0:1, gi:gi + 1])
                    g = nc.gpsimd.dma_gather(
                        zg[:, pos // 128:(pos + n) // 128, :],
                        tbl,
                        idx_sb[:, iw0 + pos // 16: iw0 + (pos + n) // 16],
                        n, cnt, cfg.RW,
                        queue_num=gq_counter[0] % N_SWDGE_Q,
                    )
                    gq_counter[0] += 1
                    add_dep_helper(g.ins, ag_insts[q],
                                   reason="gather after table ready")

                # scores: s = s_src + s_feat -> lrelu -> exp
                ss = small.tile([128, cfg.T_HALF, cfg.H], f32,
                                name=f"ss{gq}", tag="ss")
                nc.scalar.copy(ss[:], zg[:, :, cfg.HD:cfg.HDE])
                sall = small.tile([128, n_sf], f32, name=f"sall{gq}", tag="sall")
                nc.vector.tensor_tensor(
                    sall[:].rearrange("p (t h) -> p t h", h=cfg.H),
                    ss[:], sf_sb[:], ALU.add)
                slr = small.tile([128, n_sf], f32, name=f"slr{gq}", tag="slr")
                nc.vector.scalar_tensor_tensor(
                    slr[:], sall[:], 0.01, sall[:], ALU.mult, ALU.max)
                # clamp: stale rows past a trimmed gather may hold large
                # values; real scores are O(7) so 30 never binds
                nc.vector.tensor_scalar_min(slr[:], slr[:], 30.0)
                exf = small.tile([128, n_sf], f32, name=f"exf{gq}", tag="exf")
                nc.scalar.activation(exf[:], slr[:], AF.Exp)

                # msg = [ex*z | ex] in bf16
                msg = msg_pool.tile([128, cfg.T_HALF, cfg.HDE], bf,
                                    name=f"msg{gq}", tag="msg")
                nc.scalar.copy(
                    msg[:, :, cfg.HD:],
                    exf[:].rearrange("p (t h) -> p t h", h=cfg.H))
                nc.vector.tensor_tensor(
                    msg[:, :, 0:cfg.HD].rearrange("p t (h d) -> p t h d", h=cfg.H),
                    zg[:, :, 0:cfg.HD].rearrange("p t (h d) -> p t h d", h=cfg.H),
                    msg[:, :, cfg.HD:].unsqueeze(3)
                        .broadcast_to([128, cfg.T_HALF, cfg.H, cfg.D]),
                    ALU.mult,
                )

                # segment accumulate via one-hot matmul (own group per half)
                for t in range(cfg.T_HALF):
                    nc.tensor.matmul(
                        pacc[:],
                        oh[:, t, :],
                        msg[:, t, :],
                        start=(t == 0), stop=(t == cfg.T_HALF - 1),
                    )
                if q == 0:
                    # evict the A accumulator to SBUF to free the PSUM bank
                    acca = acca_pool.tile([128, cfg.HDE], f32,
                                          name=f"acca{w}", tag="acca")
                    nc.scalar.copy(acca[:], pacc[:])
                    accas[w] = acca
                    return None
                return pacc

            def finalize(w, paccb):
                acca = accas.pop(w)
                tot = small.tile([128, cfg.HDE], f32, name=f"tot{w}", tag="tot")
                nc.vector.tensor_tensor(tot[:], acca[:], paccb[:], ALU.add)
                den = small.tile([128, cfg.H], f32, name=f"den{w}", tag="den")
                nc.vector.tensor_scalar_max(den[:], tot[:, cfg.HD:], 1e-30)
                rec = small.tile([128, cfg.H], f32, name=f"rec{w}", tag="rec")
                nc.vector.reciprocal(rec[:], den[:])
                ow = outst.tile([128, cfg.HD], f32, name=f"ow{w}", tag="ow")
                nc.vector.tensor_tensor(
                    ow[:].rearrange("p (h d) -> p h d", h=cfg.H),
                    tot[:, 0:cfg.HD].rearrange("p (h d) -> p h d", h=cfg.H),
                    rec[:].unsqueeze(2).broadcast_to([128, cfg.H, cfg.D]),
                    ALU.mult,
                )
                nc.sync.dma_start(out_ext[w * 128:(w + 1) * 128, :], ow[:])

            LAG = min(5, cfg.NW)
            for step in range(cfg.NW + LAG):
                if step < cfg.NW:
                    emit_half(step, 0)
                if step >= LAG:
                    paccb = emit_half(step - LAG, 1)
                    finalize(step - LAG, paccb)

    nc.compile()
    return nc


# --------------------------------------------------------------------------
# host-side preprocessing
# --------------------------------------------------------------------------

def _remap_src(cfg: Cfg, src):
    """Map global node id -> (table, row-in-table) for the chunked tables."""
    r = src // cfg.NSH
    j = src - r * cfg.NSH
    in_a = j < cfg.ROWS_A
    row = np.where(in_a, r * cfg.ROWS_A + j,
                   r * cfg.ROWS_B + (j - cfg.ROWS_A))
    return in_a, row


def host_prep(cfg: Cfg, h, srl_emb, src, dst, W_fc, W_feat, W_attn):
    """Shard + reorder inputs; returns per-core input maps."""
    H, D = cfg.H, cfg.D

    a = np.asarray(W_attn, F32)[0]
    a_src, a_feat = a[:D], a[2 * D:3 * D]
    W_fc = np.asarray(W_fc, F32)
    Wf_eff = (np.asarray(W_feat, F32).reshape(H, D, cfg.FEAT)
              * a_feat[None, :, None]).sum(1)
    Wz_eff = (W_fc.reshape(H, D, cfg.IN_DIM) * a_src[None, :, None]).sum(1)

    Wfull = np.concatenate([W_fc.T, Wz_eff.T], axis=1)   # [IN_DIM, HDE]
    WfcT_r = np.ascontiguousarray(
        Wfull.reshape(cfg.KC, 128, cfg.HDE).transpose(1, 0, 2)).astype(BF16)
    WfeT_r = np.ascontiguousarray(Wf_eff.T).astype(BF16)
    IotaM = np.tile(np.arange(128, dtype=F32)[None, :], (128, 1)).astype(BF16)

    h_bf = np.zeros((cfg.NA_PAD, cfg.IN_DIM), BF16)
    h_bf[:cfg.NA] = np.asarray(h, F32).astype(BF16)
    srl_bf = np.asarray(srl_emb, F32).astype(BF16)

    order = np.argsort(dst, kind="stable")
    dst_s = dst[order]
    src_s = src[order]
    in_a_s, row_s = _remap_src(cfg, src_s)

    chunks = _gather_chunks(cfg.C_HALF)
    in_maps = []
    for c in range(cfg.NCORES):
        perm = np.full(cfg.E_CAP, -1, np.int64)
        dst_rel = np.full(cfg.E_CAP, -1.0, F32)
        # pads point at row 0 (gathered but masked by zero one-hot rows);
        # trimming via -1 pads + real gcnt is unsafe: untouched zg rows
        # would feed 0*NaN into the PSUM accumulate
        ia = np.zeros((cfg.NW, cfg.C_HALF), np.int16)
        ib = np.zeros((cfg.NW, cfg.C_HALF), np.int16)
        gcnt_c = np.zeros(cfg.NW * 2 * len(chunks), np.int32)
        base = c * cfg.DST_PER_CORE
        for w in range(cfg.NW):
            lo = base + w * cfg.WIN
            hi = min(base + (w + 1) * cfg.WIN, base + cfg.DST_PER_CORE)
            s0 = np.searchsorted(dst_s, lo, "left") if lo < hi else 0
            s1 = np.searchsorted(dst_s, hi, "left") if lo < hi else 0
            eid = np.arange(s0, s1)
            sel_a = eid[in_a_s[s0:s1]]
            sel_b = eid[~in_a_s[s0:s1]]
            for q, sel in ((0, sel_a), (1, sel_b)):
                k = len(sel)
                assert k <= cfg.C_HALF, f"C_HALF too small: {k}"
                pos0 = w * cfg.W_CAP + q * cfg.C_HALF
                perm[pos0:pos0 + k] = sel
                dst_rel[pos0:pos0 + k] = (dst_s[sel] - lo).astype(F32)
                arr = ia if q == 0 else ib
                arr[w, :k] = row_s[sel].astype(np.int16)
                for ci, (pos, n) in enumerate(chunks):
                    gcnt_c[(2 * w + q) * len(chunks) + ci] = n

        srl_rows = np.zeros((cfg.E_CAP, cfg.FEAT), BF16)
        valid = perm >= 0
        srl_rows[valid] = srl_bf[order[perm[valid]]]
        srlT_c = np.ascontiguousarray(srl_rows.T)

        dstrel_c = np.ascontiguousarray(
            dst_rel.reshape(-1, 128).T).astype(BF16)

        def wrap_idx(arr):  # [NW, C_HALF] -> [128, NW*C_HALF//16]
            wr = arr.reshape(cfg.NW, cfg.C_HALF // 16, 16).transpose(2, 0, 1)
            return np.ascontiguousarray(
                np.tile(wr, (8, 1, 1)).reshape(128, -1))

        hsl = h_bf[c * cfg.NSH:(c + 1) * cfg.NSH]
        hT_c = np.ascontiguousarray(
            hsl.T.reshape(cfg.KC, 128, cfg.NSH).transpose(1, 0, 2))

        in_maps.append({
            "hT": hT_c,
            "WfcT": WfcT_r,
            "WfeT": WfeT_r,
            "IotaM": IotaM,
            "srlT": srlT_c,
            "dstrel": dstrel_c,
            "idxA": wrap_idx(ia),
            "idxB": wrap_idx(ib),
            "gcnt": gcnt_c[None, :],
        })
    return in_maps


def required_c_half(cfg: Cfg, src, dst):
    """Max edges per (core, window, src-table), rounded up to 128."""
    core = dst // cfg.DST_PER_CORE
    lw = (dst - core * cfg.DST_PER_CORE) // cfg.WIN
    in_a, _ = _remap_src(cfg, src)
    key = (core * cfg.NW + lw) * 2 + (~in_a).astype(np.int64)
    counts = np.bincount(key, minlength=cfg.NCORES * cfg.NW * 2)
    return int(-(-counts.max() // 128) * 128)


# --------------------------------------------------------------------------
# entry point
# --------------------------------------------------------------------------

_CACHE = {}


def _get_graph(cfg: Cfg):
    key = (cfg.NCORES, cfg.NA_PAD, cfg.NP, cfg.E_CAP, cfg.C_HALF)
    if key not in _CACHE:
        _CACHE[key] = build_graph(cfg)
    return _CACHE[key]


def kernel(h, srl_emb, src, dst, W_fc, W_feat, W_attn, _trace=False,
           _tmpdir=None):
    src = np.asarray(src)
    dst = np.asarray(dst)
    cfg = Cfg()
    need = required_c_half(cfg, src, dst)
    if need > cfg.C_HALF:
        cfg = Cfg(c_half=need)
    nc = _get_graph(cfg)
    in_maps = host_prep(cfg, np.asarray(h), np.asarray(srl_emb), src, dst,
                        np.asarray(W_fc), np.asarray(W_feat),
                        np.asarray(W_attn))
    res = run_bass_kernel_spmd(nc, in_maps, core_ids=list(range(cfg.NCORES)),
                               trace=_trace, tmpdir=_tmpdir)
    out = np.empty((cfg.NP, cfg.H, cfg.D), F32)
    for c in range(cfg.NCORES):
        shard = np.asarray(res.results[c]["out"], F32)
        out[c * cfg.DST_PER_CORE:(c + 1) * cfg.DST_PER_CORE] = (
            shard[:cfg.DST_PER_CORE].reshape(cfg.DST_PER_CORE, cfg.H, cfg.D))
    if _trace:
        kernel._last_results = res
    return out


# revision 36
# speedup vs baseline: 1.0537x; 1.0537x over previous
"""Trainium2 distributed Bass kernel for the APGAT layer (gnn_message_passing).

Strategy (8 NeuronCores):
  - Sort edges by dst; shard dst rows (10000/8 = 1250 per core) so each pred
    node's softmax + aggregation is fully local to one core.
  - Phase A: shard the z = h @ W_fc.T matmul over nodes (6272/core, bf16).
    The matmul rhs is widened to 264 cols: [W_fc.T | Wz_eff.T], where Wz_eff
    folds a_src, so the per-node attention score s_src rides in the same
    768-byte row. The [z | s_src] table is distributed in TWO AllGathers
    (node tiles 0:25 -> table A, 25:49 -> table B) so half-A gathers can
    start while the second AllGather is still in flight.
  - Phase B: per core, 10 windows of 128 dst rows. Window edges are split by
    which table their src row lives in (both tables < 32768 rows so gather
    indices fit int16), each group padded to a fixed capacity C_half.
    Per half:
      * dma_gather [z | s_src] rows (768 B) in <=896-index chunks
        (single_packet needs <=64 ring descriptors), round-robined over
        4 SWDGE queues so multiple gathers drain concurrently
      * s_feat via PE (srl tile stationary, Wf_eff.T moving)
      * ex = exp(leaky_relu(s_src + s_feat))
      * msg = ex (x) z  (DVE broadcast multiply)
      * segment-sum via one-hot matmul: lhsT = is_equal(iota, dst_rel),
        rhs = [msg | ex]; PSUM accumulates [128 dst, 256 msg + 8 denom].
    Finalize: out = msg_sum / max(denom, tiny) -> output shard.

Softmax max-subtraction is skipped (scores are O(7); verified ~4.6e-3 rel
err vs the reference, gate is 2e-2).
"""

import sys

sys.path.insert(0, "/opt/trn_rl_repo")

import numpy as np
import ml_dtypes

import concourse.bass as bass
import concourse.bacc as bacc
import concourse.mybir as mybir
import concourse.tile as tile
from concourse.tile import add_dep_helper
from concourse.bass_utils import run_bass_kernel_spmd

BF16 = ml_dtypes.bfloat16
F32 = np.float32
AF = mybir.ActivationFunctionType
ALU = mybir.AluOpType

GATHER_CHUNK = 896   # <=64 ring descriptors per single-packet gather
N_SWDGE_Q = 4


class Cfg:
    def __init__(self, ncores=8, na=50000, np_=10000, e=400000, in_dim=512,
                 feat=128, h=8, d=32, nw=10, c_half=2816):
        self.NCORES = ncores
        self.NA = na
        self.NP = np_
        self.E = e
        self.IN_DIM = in_dim
        self.KC = in_dim // 128           # k chunks for z matmul
        self.FEAT = feat
        self.H = h
        self.D = d
        self.HD = h * d                   # 256
        self.HDE = self.HD + h            # 264: [z | s_src]
        self.RW = (-(-(self.HDE * 2) // 256) * 256) // 2  # 384 bf16 = 768 B rows
        self.NW = nw
        self.WIN = 128
        self.DST_PER_CORE = np_ // ncores
        assert self.DST_PER_CORE <= nw * 128
        nsh = -(-na // ncores)
        self.NSH = -(-nsh // 128) * 128   # nodes per core (6272)
        self.NA_PAD = self.NSH * ncores
        self.NT = self.NSH // 128         # node tiles per core (49)
        # split node tiles into two chunks == the two gather tables
        self.NT_A = -(-self.NT // 2)      # 25
        self.NT_B = self.NT - self.NT_A   # 24
        self.ROWS_A = self.NT_A * 128     # per-core rows in table A (3200)
        self.ROWS_B = self.NT_B * 128     # (3072)
        self.TBL_A = self.ROWS_A * ncores  # 25600
        self.TBL_B = self.ROWS_B * ncores  # 24576
        assert max(self.TBL_A, self.TBL_B) <= 32768  # int16 gather indices
        self.C_HALF = c_half              # per-window per-table capacity
        assert c_half % 128 == 0
        self.T_HALF = c_half // 128
        self.W_CAP = 2 * c_half
        self.T_WIN = 2 * self.T_HALF
        self.E_CAP = nw * self.W_CAP
        self.OUT_ROWS = nw * 128


def _gather_chunks(c_half):
    out, pos = [], 0
    while pos < c_half:
        n = min(GATHER_CHUNK, c_half - pos)
        out.append((pos, n))
        pos += n
    return out


def build_graph(cfg: Cfg):
    """Build the SPMD Bass graph (same graph runs on every core)."""
    nc = bacc.Bacc("TRN2", target_bir_lowering=False, debug=False,
                   num_devices=cfg.NCORES, num_swdge_queues=N_SWDGE_Q)
    bf = mybir.dt.bfloat16
    f32 = mybir.dt.float32
    i16 = mybir.dt.int16

    # ---- kernel I/O ----
    hT = nc.dram_tensor("hT", [128, cfg.KC, cfg.NSH], bf, kind="ExternalInput")
    WfcT = nc.dram_tensor("WfcT", [128, cfg.KC, cfg.HDE], bf, kind="ExternalInput")
    WfeT = nc.dram_tensor("WfeT", [cfg.FEAT, cfg.H], bf, kind="ExternalInput")
    IotaM = nc.dram_tensor("IotaM", [128, 128], bf, kind="ExternalInput")
    srlT = nc.dram_tensor("srlT", [cfg.FEAT, cfg.E_CAP], bf, kind="ExternalInput")
    dstrel = nc.dram_tensor("dstrel", [128, cfg.E_CAP // 128], bf, kind="ExternalInput")
    idxA = nc.dram_tensor("idxA", [128, cfg.NW * (cfg.C_HALF // 16)], i16, kind="ExternalInput")
    idxB = nc.dram_tensor("idxB", [128, cfg.NW * (cfg.C_HALF // 16)], i16, kind="ExternalInput")
    n_chunks = len(_gather_chunks(cfg.C_HALF))
    gcnt = nc.dram_tensor("gcnt", [1, cfg.NW * 2 * n_chunks], mybir.dt.int32,
                          kind="ExternalInput")
    out_ext = nc.dram_tensor("out", [cfg.OUT_ROWS, cfg.HD], f32, kind="ExternalOutput")

    groups = [list(range(cfg.NCORES))]
    shared = "Shared" if cfg.NCORES > 4 else "Local"

    with tile.TileContext(nc) as tc:
        with (
            tc.tile_pool(name="dram", bufs=1, space="DRAM") as dram,
            tc.tile_pool(name="consts", bufs=1) as consts,
            tc.tile_pool(name="psum_z", bufs=2, space="PSUM") as psum_z,
            tc.tile_pool(name="psum_sf", bufs=2, space="PSUM") as psum_sf_pool,
            tc.tile_pool(name="psum_acc", bufs=2, space="PSUM") as psum_acc_pool,
            tc.tile_pool(name="srl", bufs=2) as srl_pool,
            tc.tile_pool(name="zg", bufs=2) as zg_pool,
            tc.tile_pool(name="msg", bufs=2) as msg_pool,
            tc.tile_pool(name="oh", bufs=2) as oh_pool,
            tc.tile_pool(name="acca", bufs=7) as acca_pool,
            tc.tile_pool(name="sfst", bufs=6) as sfst_pool,
            tc.tile_pool(name="small", bufs=2) as small,
            tc.tile_pool(name="outst", bufs=2) as outst,
        ):
            # ---- constants / index streams (loaded once) ----
            wfe_sb = consts.tile([cfg.FEAT, cfg.H], bf)
            nc.sync.dma_start(wfe_sb[:], WfeT[:])
            iota_sb = consts.tile([128, 128], bf)
            nc.sync.dma_start(iota_sb[:], IotaM[:])
            idxa_sb = consts.tile([128, cfg.NW * (cfg.C_HALF // 16)], i16)
            nc.sync.dma_start(idxa_sb[:], idxA[:])
            idxb_sb = consts.tile([128, cfg.NW * (cfg.C_HALF // 16)], i16)
            nc.sync.dma_start(idxb_sb[:], idxB[:])
            dst_sb = consts.tile([128, cfg.E_CAP // 128], bf)
            nc.sync.dma_start(dst_sb[:], dstrel[:])
            gcnt_sb = consts.tile([1, cfg.NW * 2 * n_chunks], mybir.dt.int32)
            nc.sync.dma_start(gcnt_sb[:], gcnt[:])

            # ---- phase A: [z | s_src] shard matmuls -> two AllGathers ----
            z_out_a = dram.tile([cfg.TBL_A, cfg.RW], bf, addr_space=shared)
            z_out_b = dram.tile([cfg.TBL_B, cfg.RW], bf, addr_space=shared)
            z_in_a = dram.tile([cfg.ROWS_A, cfg.RW], bf)
            z_in_b = dram.tile([cfg.ROWS_B, cfg.RW], bf)

            ag_insts = []
            with tc.tile_pool(name="zphase", bufs=1) as zphase:
                hT_sb = zphase.tile([128, cfg.KC, cfg.NSH], bf)
                wfc_sb = zphase.tile([128, cfg.KC, cfg.HDE], bf)
                nc.sync.dma_start(wfc_sb[:], WfcT[:])
                zstage = zphase.tile([128, cfg.NT, cfg.RW], bf)
                if cfg.RW > cfg.HDE:
                    nc.vector.memset(zstage[:, :, cfg.HDE:cfg.RW], 0.0)

                for half, (t0, t1, z_in) in enumerate(
                        ((0, cfg.NT_A, z_in_a),
                         (cfg.NT_A, cfg.NT, z_in_b))):
                    nc.sync.dma_start(hT_sb[:, :, t0 * 128:t1 * 128],
                                      hT[:, :, t0 * 128:t1 * 128])
                    for nt in range(t0, t1):
                        pz = psum_z.tile([128, cfg.HDE], f32,
                                         name=f"pz{nt}", tag="pz")
                        for c in range(cfg.KC):
                            nc.tensor.matmul(
                                pz[:],
                                hT_sb[:, c, nt * 128:(nt + 1) * 128],
                                wfc_sb[:, c, :],
                                start=(c == 0), stop=(c == cfg.KC - 1),
                            )
                        nc.scalar.copy(zstage[:, nt, 0:cfg.HDE], pz[:])
                    nc.sync.dma_start(
                        z_in[:].rearrange("(t p) r -> p t r", p=128),
                        zstage[:, t0:t1, :])
                    zo = z_out_a if half == 0 else z_out_b
                    ag = nc.gpsimd.collective_compute(
                        "AllGather", ALU.bypass,
                        ins=[z_in[:].opt()], outs=[zo[:].opt()],
                        replica_groups=groups)
                    ag_insts.append(ag.ins)

            # ---- phase B: edge windows, halves software-pipelined ----
            # Half-B gathers wait on the second AllGather; emitting them LAG
            # windows behind the half-A stream keeps the Pool engine free of
            # head-of-line blocking while AG_b is still in flight. Half-A
            # PSUM accumulators are evicted to SBUF right away so the lag
            # depth is not capped by PSUM banks.
            n_sf = cfg.T_HALF * cfg.H
            gq_counter = [0]
            accas = {}

            # zero the zg slots once: rows past a trimmed gather's count keep
            # stale slot data, which must be finite for the score/exp path
            zg_init = []
            for i in range(2):
                zgi = zg_pool.tile([128, cfg.T_HALF, cfg.RW], bf,
                                   name=f"zginit{i}", tag="zg")
                nc.vector.memset(zgi[:], 0.0)
                zg_init.append(zgi)
            del zg_init

            def emit_half(w, q):
                gq = 2 * w + q
                # per-half one-hot built from the window's dst_rel columns
                oh = oh_pool.tile([128, cfg.T_HALF, 128], bf,
                                  name=f"oh{gq}", tag=f"oh{q}")
                c0 = w * cfg.T_WIN + q * cfg.T_HALF
                nc.vector.tensor_tensor(
                    oh[:],
                    iota_sb[:].unsqueeze(1).broadcast_to([128, cfg.T_HALF, 128]),
                    dst_sb[:, c0:c0 + cfg.T_HALF]
                        .unsqueeze(2).broadcast_to([128, cfg.T_HALF, 128]),
                    ALU.is_equal,
                )
                pacc = psum_acc_pool.tile([128, cfg.HDE], f32,
                                          name=f"pacc{gq}", tag=f"pacc{q}")
                srl_sb = srl_pool.tile([cfg.FEAT, cfg.C_HALF], bf,
                                       name=f"srl{gq}", tag="srl")
                nc.sync.dma_start(
                    srl_sb[:], srlT[:, gq * cfg.C_HALF:(gq + 1) * cfg.C_HALF])

                # s_feat: stationary srl tile, moving Wf_eff.T -> [128e, 8]
                psf = psum_sf_pool.tile([128, cfg.T_HALF, cfg.H], f32,
                                        name=f"psf{gq}", tag="psf")
                for t in range(cfg.T_HALF):
                    nc.tensor.matmul(
                        psf[:, t, :],
                        srl_sb[:, t * 128:(t + 1) * 128],
                        wfe_sb[:],
                        start=True, stop=True,
                    )
                sf_sb = sfst_pool.tile([128, cfg.T_HALF, cfg.H], f32,
                                       name=f"sf{gq}", tag="sf")
                nc.scalar.copy(sf_sb[:], psf[:])

                # gather [z | s_src] rows for this half, in chunks
                zg = zg_pool.tile([128, cfg.T_HALF, cfg.RW], bf,
                                  name=f"zg{gq}", tag="zg")
                idx_sb = idxa_sb if q == 0 else idxb_sb
                tbl = z_out_a[:] if q == 0 else z_out_b[:]
                iw0 = w * (cfg.C_HALF // 16)
                for ci, (pos, n) in enumerate(_gather_chunks(cfg.C_HALF)):
                    # runtime count skips the trailing -1 padding indices
                    cnt = nc.gpsimd.alloc_register(f"gcnt{gq}_{ci}")
                    gi = gq * n_chunks + ci
                    nc.gpsimd.reg_load(cnt, gcnt_sb[0:1, gi:gi + 1])
                    g = nc.gpsimd.dma_gather(
                        zg[:, pos // 128:(pos + n) // 128, :],
                        tbl,
                        idx_sb[:, iw0 + pos // 16: iw0 + (pos + n) // 16],
                        n, cnt, cfg.RW,
                        queue_num=gq_counter[0] % N_SWDGE_Q,
                    )
                    gq_counter[0] += 1
                    add_dep_helper(g.ins, ag_insts[q],
                                   reason="gather after table ready")

                # scores: s = s_src + s_feat -> lrelu -> exp
                ss = small.tile([128, cfg.T_HALF, cfg.H], f32,
                                name=f"ss{gq}", tag="ss")
                nc.scalar.copy(ss[:], zg[:, :, cfg.HD:cfg.HDE])
                sall = small.tile([128, n_sf], f32, name=f"sall{gq}", tag="sall")
                nc.vector.tensor_tensor(
                    sall[:].rearrange("p (t h) -> p t h", h=cfg.H),
                    ss[:], sf_sb[:], ALU.add)
                slr = small.tile([128, n_sf], f32, name=f"slr{gq}", tag="slr")
                nc.vector.scalar_tensor_tensor(
                    slr[:], sall[:], 0.01, sall[:], ALU.mult, ALU.max)
                # clamp: stale rows past a trimmed gather may hold large
                # values; real scores are O(7) so 30 never binds
                nc.vector.tensor_scalar_min(slr[:], slr[:], 30.0)
                exf = small.tile([128, n_sf], f32, name=f"exf{gq}", tag="exf")
                nc.scalar.activation(exf[:], slr[:], AF.Exp)

                # msg = [ex*z | ex] in bf16
                msg = msg_pool.tile([128, cfg.T_HALF, cfg.HDE], bf,
                                    name=f"msg{gq}", tag="msg")
                nc.scalar.copy(
                    msg[:, :, cfg.HD:],
                    exf[:].rearrange("p (t h) -> p t h", h=cfg.H))
                nc.vector.tensor_tensor(
                    msg[:, :, 0:cfg.HD].rearrange("p t (h d) -> p t h d", h=cfg.H),
                    zg[:, :, 0:cfg.HD].rearrange("p t (h d) -> p t h d", h=cfg.H),
                    msg[:, :, cfg.HD:].unsqueeze(3)
                        .broadcast_to([128, cfg.T_HALF, cfg.H, cfg.D]),
                    ALU.mult,
                )

                # segment accumulate via one-hot matmul (own group per half)
                for t in range(cfg.T_HALF):
                    nc.tensor.matmul(
                        pacc[:],
                        oh[:, t, :],
                        msg[:, t, :],
                        start=(t == 0), stop=(t == cfg.T_HALF - 1),
                    )
                if q == 0:
                    # evict the A accumulator to SBUF to free the PSUM bank
                    acca = acca_pool.tile([128, cfg.HDE], f32,
                                          name=f"acca{w}", tag="acca")
                    nc.scalar.copy(acca[:], pacc[:])
                    accas[w] = acca
                    return None
                return pacc

            def finalize(w, paccb):
                acca = accas.pop(w)
                tot = small.tile([128, cfg.HDE], f32, name=f"tot{w}", tag="tot")
                nc.vector.tensor_tensor(tot[:], acca[:], paccb[:], ALU.add)
                den = small.tile([128, cfg.H], f32, name=f"den{w}", tag="den")
                nc.vector.tensor_scalar_max(den[:], tot[:, cfg.HD:], 1e-30)
                rec = small.tile([128, cfg.H], f32, name=f"rec{w}", tag="rec")
                nc.vector.reciprocal(rec[:], den[:])
                ow = outst.tile([128, cfg.HD], f32, name=f"ow{w}", tag="ow")
                nc.vector.tensor_tensor(
                    ow[:].rearrange("p (h d) -> p h d", h=cfg.H),
                    tot[:, 0:cfg.HD].rearrange("p (h d) -> p h d", h=cfg.H),
                    rec[:].unsqueeze(2).broadcast_to([128, cfg.H, cfg.D]),
                    ALU.mult,
                )
                nc.sync.dma_start(out_ext[w * 128:(w + 1) * 128, :], ow[:])

            LAG = min(5, cfg.NW)
            for step in range(cfg.NW + LAG):
                if step < cfg.NW:
                    emit_half(step, 0)
                if step >= LAG:
                    paccb = emit_half(step - LAG, 1)
                    finalize(step - LAG, paccb)

    nc.compile()
    return nc


# --------------------------------------------------------------------------
# host-side preprocessing
# --------------------------------------------------------------------------

def _remap_src(cfg: Cfg, src):
    """Map global node id -> (table, row-in-table) for the chunked tables."""
    r = src // cfg.NSH
    j = src - r * cfg.NSH
    in_a = j < cfg.ROWS_A
    row = np.where(in_a, r * cfg.ROWS_A + j,
                   r * cfg.ROWS_B + (j - cfg.ROWS_A))
    return in_a, row


def host_prep(cfg: Cfg, h, srl_emb, src, dst, W_fc, W_feat, W_attn):
    """Shard + reorder inputs; returns per-core input maps."""
    H, D = cfg.H, cfg.D

    a = np.asarray(W_attn, F32)[0]
    a_src, a_feat = a[:D], a[2 * D:3 * D]
    W_fc = np.asarray(W_fc, F32)
    Wf_eff = (np.asarray(W_feat, F32).reshape(H, D, cfg.FEAT)
              * a_feat[None, :, None]).sum(1)
    Wz_eff = (W_fc.reshape(H, D, cfg.IN_DIM) * a_src[None, :, None]).sum(1)

    Wfull = np.concatenate([W_fc.T, Wz_eff.T], axis=1)   # [IN_DIM, HDE]
    WfcT_r = np.ascontiguousarray(
        Wfull.reshape(cfg.KC, 128, cfg.HDE).transpose(1, 0, 2)).astype(BF16)
    WfeT_r = np.ascontiguousarray(Wf_eff.T).astype(BF16)
    IotaM = np.tile(np.arange(128, dtype=F32)[None, :], (128, 1)).astype(BF16)

    h_bf = np.zeros((cfg.NA_PAD, cfg.IN_DIM), BF16)
    h_bf[:cfg.NA] = np.asarray(h, F32).astype(BF16)
    srl_bf = np.asarray(srl_emb, F32).astype(BF16)

    order = np.argsort(dst, kind="stable")
    dst_s = dst[order]
    src_s = src[order]
    in_a_s, row_s = _remap_src(cfg, src_s)

    chunks = _gather_chunks(cfg.C_HALF)
    in_maps = []
    for c in range(cfg.NCORES):
        perm = np.full(cfg.E_CAP, -1, np.int64)
        dst_rel = np.full(cfg.E_CAP, -1.0, F32)
        # pads point at row 0 (gathered but masked by zero one-hot rows);
        # trimming via -1 pads + real gcnt is unsafe: untouched zg rows
        # would feed 0*NaN into the PSUM accumulate
        ia = np.zeros((cfg.NW, cfg.C_HALF), np.int16)
        ib = np.zeros((cfg.NW, cfg.C_HALF), np.int16)
        gcnt_c = np.zeros(cfg.NW * 2 * len(chunks), np.int32)
        base = c * cfg.DST_PER_CORE
        for w in range(cfg.NW):
            lo = base + w * cfg.WIN
            hi = min(base + (w + 1) * cfg.WIN, base + cfg.DST_PER_CORE)
            s0 = np.searchsorted(dst_s, lo, "left") if lo < hi else 0
            s1 = np.searchsorted(dst_s, hi, "left") if lo < hi else 0
            eid = np.arange(s0, s1)
            sel_a = eid[in_a_s[s0:s1]]
            sel_b = eid[~in_a_s[s0:s1]]
            for q, sel in ((0, sel_a), (1, sel_b)):
                k = len(sel)
                assert k <= cfg.C_HALF, f"C_HALF too small: {k}"
                pos0 = w * cfg.W_CAP + q * cfg.C_HALF
                perm[pos0:pos0 + k] = sel
                dst_rel[pos0:pos0 + k] = (dst_s[sel] - lo).astype(F32)
                arr = ia if q == 0 else ib
                arr[w, :k] = row_s[sel].astype(np.int16)
                for ci, (pos, n) in enumerate(chunks):
                    gcnt_c[(2 * w + q) * len(chunks) + ci] = n

        srl_rows = np.zeros((cfg.E_CAP, cfg.FEAT), BF16)
        valid = perm >= 0
        srl_rows[valid] = srl_bf[order[perm[valid]]]
        srlT_c = np.ascontiguousarray(srl_rows.T)

        dstrel_c = np.ascontiguousarray(
            dst_rel.reshape(-1, 128).T).astype(BF16)

        def wrap_idx(arr):  # [NW, C_HALF] -> [128, NW*C_HALF//16]
            wr = arr.reshape(cfg.NW, cfg.C_HALF // 16, 16).transpose(2, 0, 1)
            return np.ascontiguousarray(
                np.tile(wr, (8, 1, 1)).reshape(128, -1))

        hsl = h_bf[c * cfg.NSH:(c + 1) * cfg.NSH]
        hT_c = np.ascontiguousarray(
            hsl.T.reshape(cfg.KC, 128, cfg.NSH).transpose(1, 0, 2))

        in_maps.append({
            "hT": hT_c,
            "WfcT": WfcT_r,
            "WfeT": WfeT_r,
            "IotaM": IotaM,
            "srlT": srlT_c,
            "dstrel": dstrel_c,
            "idxA": wrap_idx(ia),
            "idxB": wrap_idx(ib),
            "gcnt": gcnt_c[None, :],
        })
    return in_maps


def required_c_half(cfg: Cfg, src, dst):
    """Max edges per (core, window, src-table), rounded up to 128."""
    core = dst // cfg.DST_PER_CORE
    lw = (dst - core * cfg.DST_PER_CORE) // cfg.WIN
    in_a, _ = _remap_src(cfg, src)
    key = (core * cfg.NW + lw) * 2 + (~in_a).astype(np.int64)
    counts = np.bincount(key, minlength=cfg.NCORES * cfg.NW * 2)
    return int(-(-counts.max() // 128) * 128)


# --------------------------------------------------------------------------
# entry point
# --------------------------------------------------------------------------

_CACHE = {}


def _get_graph(cfg: Cfg):
    key = (cfg.NCORES, cfg.NA_PAD, cfg.NP, cfg.E_CAP, cfg.C_HALF)
    if key not in _CACHE:
        _CACHE[key] = build_graph(cfg)
    return _CACHE[key]


def kernel(h, srl_emb, src, dst, W_fc, W_feat, W_attn, _trace=False,
           _tmpdir=None):
    src = np.asarray(src)
    dst = np.asarray(dst)
    cfg = Cfg()
    need = required_c_half(cfg, src, dst)
    if need > cfg.C_HALF:
        cfg = Cfg(c_half=need)
    nc = _get_graph(cfg)
    in_maps = host_prep(cfg, np.asarray(h), np.asarray(srl_emb), src, dst,
                        np.asarray(W_fc), np.asarray(W_feat),
                        np.asarray(W_attn))
    res = run_bass_kernel_spmd(nc, in_maps, core_ids=list(range(cfg.NCORES)),
                               trace=_trace, tmpdir=_tmpdir)
    out = np.empty((cfg.NP, cfg.H, cfg.D), F32)
    for c in range(cfg.NCORES):
        shard = np.asarray(res.results[c]["out"], F32)
        out[c * cfg.DST_PER_CORE:(c + 1) * cfg.DST_PER_CORE] = (
            shard[:cfg.DST_PER_CORE].reshape(cfg.DST_PER_CORE, cfg.H, cfg.D))
    if _trace:
        kernel._last_results = res
    return out


# revision 37
# speedup vs baseline: 1.0666x; 1.0122x over previous
"""Trainium2 distributed Bass kernel for the APGAT layer (gnn_message_passing).

Strategy (8 NeuronCores):
  - Sort edges by dst; shard dst rows (10000/8 = 1250 per core) so each pred
    node's softmax + aggregation is fully local to one core.
  - Phase A: shard the z = h @ W_fc.T matmul over nodes (6272/core, bf16).
    The matmul rhs is widened to 264 cols: [W_fc.T | Wz_eff.T], where Wz_eff
    folds a_src, so the per-node attention score s_src rides in the same
    768-byte row. The [z | s_src] table is distributed in TWO AllGathers
    (node tiles 0:25 -> table A, 25:49 -> table B) so half-A gathers can
    start while the second AllGather is still in flight.
  - Phase B: per core, 10 windows of 128 dst rows. Window edges are split by
    which table their src row lives in (both tables < 32768 rows so gather
    indices fit int16), each group padded to a fixed capacity C_half.
    Per half:
      * dma_gather [z | s_src] rows (768 B) in <=896-index chunks
        (single_packet needs <=64 ring descriptors), round-robined over
        4 SWDGE queues so multiple gathers drain concurrently
      * s_feat via PE (srl tile stationary, Wf_eff.T moving)
      * ex = exp(leaky_relu(s_src + s_feat))
      * msg = ex (x) z  (DVE broadcast multiply)
      * segment-sum via one-hot matmul: lhsT = is_equal(iota, dst_rel),
        rhs = [msg | ex]; PSUM accumulates [128 dst, 256 msg + 8 denom].
    Finalize: out = msg_sum / max(denom, tiny) -> output shard.

Softmax max-subtraction is skipped (scores are O(7); verified ~4.6e-3 rel
err vs the reference, gate is 2e-2).
"""

import sys

sys.path.insert(0, "/opt/trn_rl_repo")

import numpy as np
import ml_dtypes

import concourse.bass as bass
import concourse.bacc as bacc
import concourse.mybir as mybir
import concourse.tile as tile
from concourse.tile import add_dep_helper
from concourse.bass_utils import run_bass_kernel_spmd

BF16 = ml_dtypes.bfloat16
F32 = np.float32
AF = mybir.ActivationFunctionType
ALU = mybir.AluOpType

GATHER_CHUNK = 896   # <=64 ring descriptors per single-packet gather
N_SWDGE_Q = 4


class Cfg:
    def __init__(self, ncores=8, na=50000, np_=10000, e=400000, in_dim=512,
                 feat=128, h=8, d=32, nw=10, c_half=2816):
        self.NCORES = ncores
        self.NA = na
        self.NP = np_
        self.E = e
        self.IN_DIM = in_dim
        self.KC = in_dim // 128           # k chunks for z matmul
        self.FEAT = feat
        self.H = h
        self.D = d
        self.HD = h * d                   # 256
        self.HDE = self.HD + h            # 264: [z | s_src]
        self.RW = (-(-(self.HDE * 2) // 256) * 256) // 2  # 384 bf16 = 768 B rows
        self.NW = nw
        self.WIN = 128
        self.DST_PER_CORE = np_ // ncores
        assert self.DST_PER_CORE <= nw * 128
        nsh = -(-na // ncores)
        self.NSH = -(-nsh // 128) * 128   # nodes per core (6272)
        self.NA_PAD = self.NSH * ncores
        self.NT = self.NSH // 128         # node tiles per core (49)
        # split node tiles into two chunks == the two gather tables
        self.NT_A = -(-self.NT // 2)      # 25
        self.NT_B = self.NT - self.NT_A   # 24
        self.ROWS_A = self.NT_A * 128     # per-core rows in table A (3200)
        self.ROWS_B = self.NT_B * 128     # (3072)
        self.TBL_A = self.ROWS_A * ncores  # 25600
        self.TBL_B = self.ROWS_B * ncores  # 24576
        assert max(self.TBL_A, self.TBL_B) <= 32768  # int16 gather indices
        self.C_HALF = c_half              # per-window per-table capacity
        assert c_half % 128 == 0
        self.T_HALF = c_half // 128
        self.W_CAP = 2 * c_half
        self.T_WIN = 2 * self.T_HALF
        self.E_CAP = nw * self.W_CAP
        self.OUT_ROWS = nw * 128


def _gather_chunks(c_half):
    out, pos = [], 0
    while pos < c_half:
        n = min(GATHER_CHUNK, c_half - pos)
        out.append((pos, n))
        pos += n
    return out


def build_graph(cfg: Cfg):
    """Build the SPMD Bass graph (same graph runs on every core)."""
    nc = bacc.Bacc("TRN2", target_bir_lowering=False, debug=False,
                   num_devices=cfg.NCORES, num_swdge_queues=N_SWDGE_Q)
    bf = mybir.dt.bfloat16
    f32 = mybir.dt.float32
    i16 = mybir.dt.int16

    # ---- kernel I/O ----
    hT = nc.dram_tensor("hT", [128, cfg.KC, cfg.NSH], bf, kind="ExternalInput")
    WfcT = nc.dram_tensor("WfcT", [128, cfg.KC, cfg.HDE], bf, kind="ExternalInput")
    WfeT = nc.dram_tensor("WfeT", [cfg.FEAT, cfg.H], bf, kind="ExternalInput")
    IotaM = nc.dram_tensor("IotaM", [128, 128], bf, kind="ExternalInput")
    srlT = nc.dram_tensor("srlT", [cfg.FEAT, cfg.E_CAP], bf, kind="ExternalInput")
    dstrel = nc.dram_tensor("dstrel", [128, cfg.E_CAP // 128], bf, kind="ExternalInput")
    idxA = nc.dram_tensor("idxA", [128, cfg.NW * (cfg.C_HALF // 16)], i16, kind="ExternalInput")
    idxB = nc.dram_tensor("idxB", [128, cfg.NW * (cfg.C_HALF // 16)], i16, kind="ExternalInput")
    n_chunks = len(_gather_chunks(cfg.C_HALF))
    gcnt = nc.dram_tensor("gcnt", [1, cfg.NW * 2 * n_chunks], mybir.dt.int32,
                          kind="ExternalInput")
    out_ext = nc.dram_tensor("out", [cfg.OUT_ROWS, cfg.HD], f32, kind="ExternalOutput")

    groups = [list(range(cfg.NCORES))]
    shared = "Shared" if cfg.NCORES > 4 else "Local"

    with tile.TileContext(nc) as tc:
        with (
            tc.tile_pool(name="dram", bufs=1, space="DRAM") as dram,
            tc.tile_pool(name="consts", bufs=1) as consts,
            tc.tile_pool(name="psum_z", bufs=2, space="PSUM") as psum_z,
            tc.tile_pool(name="psum_sf", bufs=2, space="PSUM") as psum_sf_pool,
            tc.tile_pool(name="psum_acc", bufs=2, space="PSUM") as psum_acc_pool,
            tc.tile_pool(name="srl", bufs=2) as srl_pool,
            tc.tile_pool(name="zg", bufs=2) as zg_pool,
            tc.tile_pool(name="msg", bufs=2) as msg_pool,
            tc.tile_pool(name="oh", bufs=2) as oh_pool,
            tc.tile_pool(name="acca", bufs=8) as acca_pool,
            tc.tile_pool(name="sfst", bufs=6) as sfst_pool,
            tc.tile_pool(name="small", bufs=2) as small,
            tc.tile_pool(name="outst", bufs=2) as outst,
        ):
            # ---- constants / index streams (loaded once) ----
            wfe_sb = consts.tile([cfg.FEAT, cfg.H], bf)
            nc.sync.dma_start(wfe_sb[:], WfeT[:])
            iota_sb = consts.tile([128, 128], bf)
            nc.sync.dma_start(iota_sb[:], IotaM[:])
            idxa_sb = consts.tile([128, cfg.NW * (cfg.C_HALF // 16)], i16)
            nc.sync.dma_start(idxa_sb[:], idxA[:])
            idxb_sb = consts.tile([128, cfg.NW * (cfg.C_HALF // 16)], i16)
            nc.sync.dma_start(idxb_sb[:], idxB[:])
            dst_sb = consts.tile([128, cfg.E_CAP // 128], bf)
            nc.sync.dma_start(dst_sb[:], dstrel[:])
            gcnt_sb = consts.tile([1, cfg.NW * 2 * n_chunks], mybir.dt.int32)
            nc.sync.dma_start(gcnt_sb[:], gcnt[:])

            # ---- phase A: [z | s_src] shard matmuls -> two AllGathers ----
            z_out_a = dram.tile([cfg.TBL_A, cfg.RW], bf, addr_space=shared)
            z_out_b = dram.tile([cfg.TBL_B, cfg.RW], bf, addr_space=shared)
            z_in_a = dram.tile([cfg.ROWS_A, cfg.RW], bf)
            z_in_b = dram.tile([cfg.ROWS_B, cfg.RW], bf)

            ag_insts = []
            with tc.tile_pool(name="zphase", bufs=1) as zphase:
                hT_sb = zphase.tile([128, cfg.KC, cfg.NSH], bf)
                wfc_sb = zphase.tile([128, cfg.KC, cfg.HDE], bf)
                nc.sync.dma_start(wfc_sb[:], WfcT[:])
                zstage = zphase.tile([128, cfg.NT, cfg.RW], bf)
                if cfg.RW > cfg.HDE:
                    nc.vector.memset(zstage[:, :, cfg.HDE:cfg.RW], 0.0)

                for half, (t0, t1, z_in) in enumerate(
                        ((0, cfg.NT_A, z_in_a),
                         (cfg.NT_A, cfg.NT, z_in_b))):
                    nc.sync.dma_start(hT_sb[:, :, t0 * 128:t1 * 128],
                                      hT[:, :, t0 * 128:t1 * 128])
                    for nt in range(t0, t1):
                        pz = psum_z.tile([128, cfg.HDE], f32,
                                         name=f"pz{nt}", tag="pz")
                        for c in range(cfg.KC):
                            nc.tensor.matmul(
                                pz[:],
                                hT_sb[:, c, nt * 128:(nt + 1) * 128],
                                wfc_sb[:, c, :],
                                start=(c == 0), stop=(c == cfg.KC - 1),
                            )
                        nc.scalar.copy(zstage[:, nt, 0:cfg.HDE], pz[:])
                    nc.sync.dma_start(
                        z_in[:].rearrange("(t p) r -> p t r", p=128),
                        zstage[:, t0:t1, :])
                    zo = z_out_a if half == 0 else z_out_b
                    ag = nc.gpsimd.collective_compute(
                        "AllGather", ALU.bypass,
                        ins=[z_in[:].opt()], outs=[zo[:].opt()],
                        replica_groups=groups)
                    ag_insts.append(ag.ins)

            # ---- phase B: edge windows, halves software-pipelined ----
            # Half-B gathers wait on the second AllGather; emitting them LAG
            # windows behind the half-A stream keeps the Pool engine free of
            # head-of-line blocking while AG_b is still in flight. Half-A
            # PSUM accumulators are evicted to SBUF right away so the lag
            # depth is not capped by PSUM banks.
            n_sf = cfg.T_HALF * cfg.H
            gq_counter = [0]
            accas = {}

            # zero the zg slots once: rows past a trimmed gather's count keep
            # stale slot data, which must be finite for the score/exp path
            zg_init = []
            for i in range(2):
                zgi = zg_pool.tile([128, cfg.T_HALF, cfg.RW], bf,
                                   name=f"zginit{i}", tag="zg")
                nc.vector.memset(zgi[:], 0.0)
                zg_init.append(zgi)
            del zg_init

            def emit_half(w, q):
                gq = 2 * w + q
                # per-half one-hot built from the window's dst_rel columns
                oh = oh_pool.tile([128, cfg.T_HALF, 128], bf,
                                  name=f"oh{gq}", tag=f"oh{q}")
                c0 = w * cfg.T_WIN + q * cfg.T_HALF
                nc.vector.tensor_tensor(
                    oh[:],
                    iota_sb[:].unsqueeze(1).broadcast_to([128, cfg.T_HALF, 128]),
                    dst_sb[:, c0:c0 + cfg.T_HALF]
                        .unsqueeze(2).broadcast_to([128, cfg.T_HALF, 128]),
                    ALU.is_equal,
                )
                pacc = psum_acc_pool.tile([128, cfg.HDE], f32,
                                          name=f"pacc{gq}", tag=f"pacc{q}")
                srl_sb = srl_pool.tile([cfg.FEAT, cfg.C_HALF], bf,
                                       name=f"srl{gq}", tag="srl")
                nc.sync.dma_start(
                    srl_sb[:], srlT[:, gq * cfg.C_HALF:(gq + 1) * cfg.C_HALF])

                # s_feat: stationary srl tile, moving Wf_eff.T -> [128e, 8]
                psf = psum_sf_pool.tile([128, cfg.T_HALF, cfg.H], f32,
                                        name=f"psf{gq}", tag="psf")
                for t in range(cfg.T_HALF):
                    nc.tensor.matmul(
                        psf[:, t, :],
                        srl_sb[:, t * 128:(t + 1) * 128],
                        wfe_sb[:],
                        start=True, stop=True,
                    )
                sf_sb = sfst_pool.tile([128, cfg.T_HALF, cfg.H], f32,
                                       name=f"sf{gq}", tag="sf")
                nc.scalar.copy(sf_sb[:], psf[:])

                # gather [z | s_src] rows for this half, in chunks
                zg = zg_pool.tile([128, cfg.T_HALF, cfg.RW], bf,
                                  name=f"zg{gq}", tag="zg")
                idx_sb = idxa_sb if q == 0 else idxb_sb
                tbl = z_out_a[:] if q == 0 else z_out_b[:]
                iw0 = w * (cfg.C_HALF // 16)
                for ci, (pos, n) in enumerate(_gather_chunks(cfg.C_HALF)):
                    # runtime count skips the trailing -1 padding indices
                    cnt = nc.gpsimd.alloc_register(f"gcnt{gq}_{ci}")
                    gi = gq * n_chunks + ci
                    nc.gpsimd.reg_load(cnt, gcnt_sb[0:1, gi:gi + 1])
                    g = nc.gpsimd.dma_gather(
                        zg[:, pos // 128:(pos + n) // 128, :],
                        tbl,
                        idx_sb[:, iw0 + pos // 16: iw0 + (pos + n) // 16],
                        n, cnt, cfg.RW,
                        queue_num=gq_counter[0] % N_SWDGE_Q,
                    )
                    gq_counter[0] += 1
                    add_dep_helper(g.ins, ag_insts[q],
                                   reason="gather after table ready")

                # scores: s = s_src + s_feat -> lrelu -> exp
                ss = small.tile([128, cfg.T_HALF, cfg.H], f32,
                                name=f"ss{gq}", tag="ss")
                nc.scalar.copy(ss[:], zg[:, :, cfg.HD:cfg.HDE])
                sall = small.tile([128, n_sf], f32, name=f"sall{gq}", tag="sall")
                nc.vector.tensor_tensor(
                    sall[:].rearrange("p (t h) -> p t h", h=cfg.H),
                    ss[:], sf_sb[:], ALU.add)
                slr = small.tile([128, n_sf], f32, name=f"slr{gq}", tag="slr")
                nc.vector.scalar_tensor_tensor(
                    slr[:], sall[:], 0.01, sall[:], ALU.mult, ALU.max)
                # clamp: stale rows past a trimmed gather may hold large
                # values; real scores are O(7) so 30 never binds
                nc.vector.tensor_scalar_min(slr[:], slr[:], 30.0)
                exf = small.tile([128, n_sf], f32, name=f"exf{gq}", tag="exf")
                nc.scalar.activation(exf[:], slr[:], AF.Exp)

                # msg = [ex*z | ex] in bf16
                msg = msg_pool.tile([128, cfg.T_HALF, cfg.HDE], bf,
                                    name=f"msg{gq}", tag="msg")
                nc.scalar.copy(
                    msg[:, :, cfg.HD:],
                    exf[:].rearrange("p (t h) -> p t h", h=cfg.H))
                nc.vector.tensor_tensor(
                    msg[:, :, 0:cfg.HD].rearrange("p t (h d) -> p t h d", h=cfg.H),
                    zg[:, :, 0:cfg.HD].rearrange("p t (h d) -> p t h d", h=cfg.H),
                    msg[:, :, cfg.HD:].unsqueeze(3)
                        .broadcast_to([128, cfg.T_HALF, cfg.H, cfg.D]),
                    ALU.mult,
                )

                # segment accumulate via one-hot matmul (own group per half)
                for t in range(cfg.T_HALF):
                    nc.tensor.matmul(
                        pacc[:],
                        oh[:, t, :],
                        msg[:, t, :],
                        start=(t == 0), stop=(t == cfg.T_HALF - 1),
                    )
                if q == 0:
                    # evict the A accumulator to SBUF to free the PSUM bank
                    acca = acca_pool.tile([128, cfg.HDE], f32,
                                          name=f"acca{w}", tag="acca")
                    nc.scalar.copy(acca[:], pacc[:])
                    accas[w] = acca
                    return None
                return pacc

            def finalize(w, paccb):
                acca = accas.pop(w)
                tot = small.tile([128, cfg.HDE], f32, name=f"tot{w}", tag="tot")
                nc.vector.tensor_tensor(tot[:], acca[:], paccb[:], ALU.add)
                den = small.tile([128, cfg.H], f32, name=f"den{w}", tag="den")
                nc.vector.tensor_scalar_max(den[:], tot[:, cfg.HD:], 1e-30)
                rec = small.tile([128, cfg.H], f32, name=f"rec{w}", tag="rec")
                nc.vector.reciprocal(rec[:], den[:])
                ow = outst.tile([128, cfg.HD], f32, name=f"ow{w}", tag="ow")
                nc.vector.tensor_tensor(
                    ow[:].rearrange("p (h d) -> p h d", h=cfg.H),
                    tot[:, 0:cfg.HD].rearrange("p (h d) -> p h d", h=cfg.H),
                    rec[:].unsqueeze(2).broadcast_to([128, cfg.H, cfg.D]),
                    ALU.mult,
                )
                nc.sync.dma_start(out_ext[w * 128:(w + 1) * 128, :], ow[:])

            LAG = min(6, cfg.NW)
            for step in range(cfg.NW + LAG):
                if step < cfg.NW:
                    emit_half(step, 0)
                if step >= LAG:
                    paccb = emit_half(step - LAG, 1)
                    finalize(step - LAG, paccb)

    nc.compile()
    return nc


# --------------------------------------------------------------------------
# host-side preprocessing
# --------------------------------------------------------------------------

def _remap_src(cfg: Cfg, src):
    """Map global node id -> (table, row-in-table) for the chunked tables."""
    r = src // cfg.NSH
    j = src - r * cfg.NSH
    in_a = j < cfg.ROWS_A
    row = np.where(in_a, r * cfg.ROWS_A + j,
                   r * cfg.ROWS_B + (j - cfg.ROWS_A))
    return in_a, row


def host_prep(cfg: Cfg, h, srl_emb, src, dst, W_fc, W_feat, W_attn):
    """Shard + reorder inputs; returns per-core input maps."""
    H, D = cfg.H, cfg.D

    a = np.asarray(W_attn, F32)[0]
    a_src, a_feat = a[:D], a[2 * D:3 * D]
    W_fc = np.asarray(W_fc, F32)
    Wf_eff = (np.asarray(W_feat, F32).reshape(H, D, cfg.FEAT)
              * a_feat[None, :, None]).sum(1)
    Wz_eff = (W_fc.reshape(H, D, cfg.IN_DIM) * a_src[None, :, None]).sum(1)

    Wfull = np.concatenate([W_fc.T, Wz_eff.T], axis=1)   # [IN_DIM, HDE]
    WfcT_r = np.ascontiguousarray(
        Wfull.reshape(cfg.KC, 128, cfg.HDE).transpose(1, 0, 2)).astype(BF16)
    WfeT_r = np.ascontiguousarray(Wf_eff.T).astype(BF16)
    IotaM = np.tile(np.arange(128, dtype=F32)[None, :], (128, 1)).astype(BF16)

    h_bf = np.zeros((cfg.NA_PAD, cfg.IN_DIM), BF16)
    h_bf[:cfg.NA] = np.asarray(h, F32).astype(BF16)
    srl_bf = np.asarray(srl_emb, F32).astype(BF16)

    order = np.argsort(dst, kind="stable")
    dst_s = dst[order]
    src_s = src[order]
    in_a_s, row_s = _remap_src(cfg, src_s)

    chunks = _gather_chunks(cfg.C_HALF)
    in_maps = []
    for c in range(cfg.NCORES):
        perm = np.full(cfg.E_CAP, -1, np.int64)
        dst_rel = np.full(cfg.E_CAP, -1.0, F32)
        # pads point at row 0 (gathered but masked by zero one-hot rows);
        # trimming via -1 pads + real gcnt is unsafe: untouched zg rows
        # would feed 0*NaN into the PSUM accumulate
        ia = np.zeros((cfg.NW, cfg.C_HALF), np.int16)
        ib = np.zeros((cfg.NW, cfg.C_HALF), np.int16)
        gcnt_c = np.zeros(cfg.NW * 2 * len(chunks), np.int32)
        base = c * cfg.DST_PER_CORE
        for w in range(cfg.NW):
            lo = base + w * cfg.WIN
            hi = min(base + (w + 1) * cfg.WIN, base + cfg.DST_PER_CORE)
            s0 = np.searchsorted(dst_s, lo, "left") if lo < hi else 0
            s1 = np.searchsorted(dst_s, hi, "left") if lo < hi else 0
            eid = np.arange(s0, s1)
            sel_a = eid[in_a_s[s0:s1]]
            sel_b = eid[~in_a_s[s0:s1]]
            for q, sel in ((0, sel_a), (1, sel_b)):
                k = len(sel)
                assert k <= cfg.C_HALF, f"C_HALF too small: {k}"
                pos0 = w * cfg.W_CAP + q * cfg.C_HALF
                perm[pos0:pos0 + k] = sel
                dst_rel[pos0:pos0 + k] = (dst_s[sel] - lo).astype(F32)
                arr = ia if q == 0 else ib
                arr[w, :k] = row_s[sel].astype(np.int16)
                for ci, (pos, n) in enumerate(chunks):
                    gcnt_c[(2 * w + q) * len(chunks) + ci] = n

        srl_rows = np.zeros((cfg.E_CAP, cfg.FEAT), BF16)
        valid = perm >= 0
        srl_rows[valid] = srl_bf[order[perm[valid]]]
        srlT_c = np.ascontiguousarray(srl_rows.T)

        dstrel_c = np.ascontiguousarray(
            dst_rel.reshape(-1, 128).T).astype(BF16)

        def wrap_idx(arr):  # [NW, C_HALF] -> [128, NW*C_HALF//16]
            wr = arr.reshape(cfg.NW, cfg.C_HALF // 16, 16).transpose(2, 0, 1)
            return np.ascontiguousarray(
                np.tile(wr, (8, 1, 1)).reshape(128, -1))

        hsl = h_bf[c * cfg.NSH:(c + 1) * cfg.NSH]
        hT_c = np.ascontiguousarray(
            hsl.T.reshape(cfg.KC, 128, cfg.NSH).transpose(1, 0, 2))

        in_maps.append({
            "hT": hT_c,
            "WfcT": WfcT_r,
            "WfeT": WfeT_r,
            "IotaM": IotaM,
            "srlT": srlT_c,
            "dstrel": dstrel_c,
            "idxA": wrap_idx(ia),
            "idxB": wrap_idx(ib),
            "gcnt": gcnt_c[None, :],
        })
    return in_maps


def required_c_half(cfg: Cfg, src, dst):
    """Max edges per (core, window, src-table), rounded up to 128."""
    core = dst // cfg.DST_PER_CORE
    lw = (dst - core * cfg.DST_PER_CORE) // cfg.WIN
    in_a, _ = _remap_src(cfg, src)
    key = (core * cfg.NW + lw) * 2 + (~in_a).astype(np.int64)
    counts = np.bincount(key, minlength=cfg.NCORES * cfg.NW * 2)
    return int(-(-counts.max() // 128) * 128)


# --------------------------------------------------------------------------
# entry point
# --------------------------------------------------------------------------

_CACHE = {}


def _get_graph(cfg: Cfg):
    key = (cfg.NCORES, cfg.NA_PAD, cfg.NP, cfg.E_CAP, cfg.C_HALF)
    if key not in _CACHE:
        _CACHE[key] = build_graph(cfg)
    return _CACHE[key]


def kernel(h, srl_emb, src, dst, W_fc, W_feat, W_attn, _trace=False,
           _tmpdir=None):
    src = np.asarray(src)
    dst = np.asarray(dst)
    cfg = Cfg()
    need = required_c_half(cfg, src, dst)
    if need > cfg.C_HALF:
        cfg = Cfg(c_half=need)
    nc = _get_graph(cfg)
    in_maps = host_prep(cfg, np.asarray(h), np.asarray(srl_emb), src, dst,
                        np.asarray(W_fc), np.asarray(W_feat),
                        np.asarray(W_attn))
    res = run_bass_kernel_spmd(nc, in_maps, core_ids=list(range(cfg.NCORES)),
                               trace=_trace, tmpdir=_tmpdir)
    out = np.empty((cfg.NP, cfg.H, cfg.D), F32)
    for c in range(cfg.NCORES):
        shard = np.asarray(res.results[c]["out"], F32)
        out[c * cfg.DST_PER_CORE:(c + 1) * cfg.DST_PER_CORE] = (
            shard[:cfg.DST_PER_CORE].reshape(cfg.DST_PER_CORE, cfg.H, cfg.D))
    if _trace:
        kernel._last_results = res
    return out
